# revision 1
# baseline (speedup 1.0000x reference)
"""CrossNetMoE forward on 8 Trainium2 NeuronCores (Bass/Tile).

Math (per layer i, E=4 experts, rank R=64, D=1024):
    v = tanh(V_e @ xl)            [B,E,R]
    c = tanh(C_e @ v_e)           [B,E,R]
    g = softmax(Wg_e . xl)        [B,E]
    u = sum_e (g_e * c_e) @ U_e.T + b      (softmax weights sum to 1)
    xl' = tanh(u * x0 + xl)   (last layer: no tanh)

Strategy: pure data-parallel over batch (2048 rows/core), everything kept in
transposed layout [D, B] on-chip so each layer's matmuls consume the previous
layer's output directly as the PE moving operand; softmax over the 4 experts is
done with tiny auxiliary matmuls (partition reduction / broadcast); the final
`u * x0 + xl` keeps u in a bf16 PSUM bank: DVE multiplies x0 and adds xl in
place (both at 2x 16-bit DVE rate), ACT evacuates with the tanh. The last
layer's result is written bf16 straight from the DVE add and DMA'd per half on
the otherwise-idle Pool DMA queue.

Host side transposes x once and un-transposes the result; weights are packed
host-side into SBUF-image blobs replicated to all cores.
"""
import json
import os
import sys

sys.path.insert(0, "/opt/trn_rl_repo")

import numpy as np

L, E, D, R = 3, 4, 1024, 64
B = 16384
NCORES = 8
BC = B // NCORES          # 2048 rows per core
N = 256                   # batch columns per group (matmul moving free dim)
G = BC // N               # 8 groups per core
NCH = D // 128            # 8 d-chunks

# per-layer weight blob column offsets (f32 columns, [128, COLS_L])
V_OFF = 0                 # 2 groups x 8 chunks x 128
U_OFF = 2048              # 2 kchunks x 8 mchunks x 128
C_OFF = 4096              # 2 groups x 128
W_OFF = 4352              # 8 chunks x 4
B_OFF = 4384              # 8 chunks x 1
COLS_L = 4392
# common blob
I_OFF = 0                 # identity 128
P_OFF = 128               # 2 groups x 128 (partitions 0..3)
O4_OFF = 384              # ones [4,1]
O14_OFF = 385             # ones [1,4]
O44_OFF = 389             # all-ones [4,4] for Z4
COLS_C = 396

_EXEMPT = {"Call"}


def _legalize_json_bytes(raw: bytes) -> bytes:
    """Split multi-wait instructions: walrus allows 1 sync-wait per inst."""
    m = json.loads(raw)
    counter = [0]

    def fix_block(block):
        insts = block.get("instructions")
        if insts is not None:
            out = []
            for inst in insts:
                si = inst.get("sync_info")
                if (
                    si
                    and inst.get("opcode") not in _EXEMPT
                    and len(si.get("on_wait") or []) > 1
                ):
                    for w in si["on_wait"][:-1]:
                        counter[0] += 1
                        out.append(
                            {
                                "name": f"I-waitsplit-{counter[0]}",
                                "opcode": "NoOp",
                                "engine": inst["engine"],
                                "ins": [],
                                "outs": [],
                                "debug": 0,
                                "sync_info": {"on_wait": [w], "on_update": []},
                            }
                        )
                    si["on_wait"] = [si["on_wait"][-1]]
                out.append(inst)
            block["instructions"] = out
        for sub in block.get("blocks") or []:
            fix_block(sub)

    for f in m["functions"]:
        for b in f["blocks"]:
            fix_block(b)
    return json.dumps(m).encode()


def pack_weights(U, V, C, Wg, b):
    """Pack parameters into SBUF-image blobs [128, COLS] (host side)."""
    U, V, C, Wg, b = (np.asarray(a, np.float32) for a in (U, V, C, Wg, b))
    blobs = {}
    p = np.arange(128)
    for l in range(L):
        blob = np.zeros((128, COLS_L), np.float32)
        # Vw: lhsT chunk for vT matmuls: [p=d%128, (g,c,m)]; m -> (e=2g+m//64, r=m%64)
        for g in range(2):
            for c in range(NCH):
                m = np.arange(128)
                # V[l, 2g + m//64, m%64, c*128+p]
                blob[:, V_OFF + (g * 8 + c) * 128 : V_OFF + (g * 8 + c + 1) * 128] = (
                    V[l, 2 * g + m[None, :] // 64, m[None, :] % 64, c * 128 + p[:, None]]
                )
        # Uw: [p=(e,r2)%128 of kchunk, (kc,mc,q)] = U[l, (kc*128+p)//64, mc*128+q, (kc*128+p)%64]
        for kc in range(2):
            for mc in range(NCH):
                q = np.arange(128)
                blob[:, U_OFF + (kc * 8 + mc) * 128 : U_OFF + (kc * 8 + mc + 1) * 128] = (
                    U[l, (kc * 128 + p[:, None]) // 64, mc * 128 + q[None, :], (kc * 128 + p[:, None]) % 64]
                )
        # Cw: blockdiag pairs: [p=(el',r1), (g,j=(el,r2))]
        for g in range(2):
            j = np.arange(128)
            el_p = p[:, None] // 64
            el_j = j[None, :] // 64
            val = C[l, 2 * g + el_j, j[None, :] % 64, p[:, None] % 64]
            blob[:, C_OFF + g * 128 : C_OFF + (g + 1) * 128] = np.where(el_p == el_j, val, 0.0)
        # Ww: [p=d%128, (c,e)]
        for c in range(NCH):
            blob[:, W_OFF + c * 4 : W_OFF + (c + 1) * 4] = Wg[l, :, c * 128 + p]
        # bias
        for c in range(NCH):
            blob[:, B_OFF + c] = b[l, c * 128 + p]
        blobs[f"wl{l}"] = blob
    wc = np.zeros((128, COLS_C), np.float32)
    wc[:, I_OFF : I_OFF + 128] = np.eye(128, dtype=np.float32)
    for g in range(2):
        m = np.arange(128)
        wc[0:4, P_OFF + g * 128 : P_OFF + (g + 1) * 128] = (
            np.arange(4)[:, None] == (2 * g + m[None, :] // 64)
        ).astype(np.float32)
    wc[0:4, O4_OFF] = 1.0
    wc[0:1, O14_OFF : O14_OFF + 4] = 1.0
    wc[0:4, O44_OFF : O44_OFF + 4] = 1.0
    blobs["wc"] = wc
    return blobs


def build_nc(bias_nonzero=False, mode="bf16"):
    import concourse.bass as bass
    import concourse.tile as tile
    from concourse import mybir
    from concourse.tile import add_dep_helper

    f32 = mybir.dt.float32
    AF = mybir.ActivationFunctionType
    ALU = mybir.AluOpType
    bf16 = mybir.dt.bfloat16
    mt = bf16

    nc = bass.Bass()
    xT = nc.dram_tensor("xT", [D, BC], mt, kind="ExternalInput")
    wl = [nc.dram_tensor(f"wl{l}", [128, COLS_L], mt, kind="ExternalInput") for l in range(L)]
    wbd = nc.dram_tensor("wb", [128, L * NCH], f32, kind="ExternalInput") if bias_nonzero else None
    wid = nc.dram_tensor("wid", [128, 388], mt, kind="ExternalInput")
    outT = nc.dram_tensor("outT", [D, BC], mt, kind="ExternalOutput")

    # chain matmuls that share a psum tile so scheduler keeps program order
    last_mm = {}

    def mm(key, out, lhsT, rhs, start, stop):
        inst = nc.tensor.matmul(out, lhsT, rhs, start=start, stop=stop, skip_group_check=True)
        if key in last_mm:
            add_dep_helper(inst.ins, last_mm[key].ins, sync=False, reason="psum order")
        last_mm[key] = inst
        return inst

    with tile.TileContext(nc) as tc:
        with (
            tc.tile_pool(name="wpool", bufs=1) as wpool,
            tc.tile_pool(name="xpool", bufs=1) as xpool,
            tc.tile_pool(name="mid", bufs=1) as mid,
            tc.tile_pool(name="pspool", bufs=1, space="PSUM") as pspool,
            nc.allow_low_precision(reason="bf16 matmul/psum pipeline (intentional)"),
        ):
            wt = []
            for l in range(L):
                w_l = wpool.tile([128, COLS_L], mt, name=f"wt{l}")
                wt.append(w_l)
            wb_f32 = None
            identm = wpool.tile([128, 388], mt, name="identm")
            Pb = identm[0:4, 128:384]
            ones44 = identm[0:4, 384:388]

            xT_v = xT.rearrange("(c p) b -> p c b", p=128)
            outT_h = outT.rearrange("(h c p) b -> p h c b", p=128, h=2)

            ROT = 4  # groups processed in rotation
            for sup in range(G // ROT):
                gs = [ROT * sup + i for i in range(ROT)]
                st = {}
                if sup == 0:
                    # startup: V weights + first x0 group first so PE can start
                    nc.sync.dma_start(wt[0][:, 0:2048], wl[0][:, 0:2048])
                    nc.scalar.dma_start(identm[:], wid[:])
                    for i, g in enumerate(gs):
                        x0t = xpool.tile([128, NCH * N], mt, tag="x0", bufs=8, name=f"x0_{g}")
                        (nc.sync if i == 0 else nc.scalar).dma_start(
                            x0t[:], xT_v[:, :, g * N : (g + 1) * N]
                        )
                        if i == 0:
                            # gating/C/bias columns (small, needed early)
                            nc.sync.dma_start(wt[0][:, 4096:COLS_L], wl[0][:, 4096:COLS_L])
                        st[g] = dict(x0t=x0t, xin=x0t)
                    nc.sync.dma_start(wt[0][:, 2048:4096], wl[0][:, 2048:4096])
                    if bias_nonzero:
                        wbt = wpool.tile([128, L * NCH], f32)
                        nc.sync.dma_start(wbt[:], wbd[:])
                        wb_f32 = [wbt[:, l * NCH : (l + 1) * NCH] for l in range(L)]
                    for l_ in range(1, L):
                        nc.scalar.dma_start(wt[l_][:], wl[l_][:])
                else:
                    for g in gs:
                        x0t = xpool.tile([128, NCH * N], mt, tag="x0", bufs=8, name=f"x0_{g}")
                        nc.sync.dma_start(x0t[:], xT_v[:, :, g * N : (g + 1) * N])
                        st[g] = dict(x0t=x0t, xin=x0t)

                for l in range(L):
                    # ---------- P1 + per-group softmax chain ----------
                    for gi, g in enumerate(gs):
                        S = st[g]
                        xin = S["xin"]
                        kv = f"v{g}_{l}"
                        v_ps = pspool.tile([128, 512], f32, tag="v", bufs=2, name=f"vps{g}_{l}")
                        for c in range(NCH):
                            rhs = xin[:, c * N : (c + 1) * N]
                            mm(kv, v_ps[:, 0:N], wt[l][:, V_OFF + c * 128 : V_OFF + (c + 1) * 128], rhs, start=(c == 0), stop=False)
                            mm(kv, v_ps[:, N : 2 * N], wt[l][:, V_OFF + (8 + c) * 128 : V_OFF + (9 + c) * 128], rhs, start=False, stop=(c == NCH - 1))
                        # gate logits in transposed [batch_p, expert] layout:
                        # 4-col matmuls with xin chunk-halves as the stationary side
                        ks = f"sb{g}_{l}"
                        sbt = pspool.tile([128, 8], f32, tag="sbt", bufs=1, name=f"sbt{g}_{l}")
                        for h in range(2):
                            for c in range(NCH):
                                mm(
                                    ks,
                                    sbt[:, h * 4 : (h + 1) * 4],
                                    xin[:, c * N + h * 128 : c * N + h * 128 + 128],
                                    wt[l][:, W_OFF + c * 4 : W_OFF + (c + 1) * 4],
                                    start=(c == 0),
                                    stop=(c == NCH - 1),
                                )
                        # chain: tanh(v); softmax over the 4-col free dim
                        vt = mid.tile([128, 512], mt, tag="vt", bufs=5, name=f"vt{g}_{l}")
                        nc.scalar.activation(vt[:], v_ps[:], AF.Tanh)
                        ebt = mid.tile([128, 8], mt, tag="ebt", bufs=5, name=f"ebt{g}_{l}")
                        zsum = mid.tile([128, 2], f32, tag="zs", bufs=5, name=f"zs{g}_{l}")
                        for h in range(2):
                            nc.scalar.activation(
                                ebt[:, h * 4 : (h + 1) * 4],
                                sbt[:, h * 4 : (h + 1) * 4],
                                AF.Exp,
                                accum_out=zsum[:, h : h + 1],
                            )
                        rz = mid.tile([128, 2], f32, tag="rz", bufs=5, name=f"rz{g}_{l}")
                        nc.vector.reciprocal(rz[:], zsum[:])
                        gbt = mid.tile([128, 8], mt, tag="gbt", bufs=5, name=f"gbt{g}_{l}")
                        for h in range(2):
                            nc.vector.tensor_scalar_mul(
                                gbt[:, h * 4 : (h + 1) * 4],
                                ebt[:, h * 4 : (h + 1) * 4],
                                rz[:, h : h + 1],
                            )
                        # transpose gates back to [expert, batch] (bf16 PSUM out)
                        kt = f"gt{g}_{l}"
                        gt = pspool.tile([4, 256], mt, tag="gt", bufs=1, name=f"gt{g}_{l}")
                        for h in range(2):
                            inst = nc.tensor.transpose(
                                gt[0:4, h * 128 : (h + 1) * 128],
                                gbt[:, h * 4 : (h + 1) * 4],
                                identm[:, 0:128],
                            )
                            if kt in last_mm:
                                add_dep_helper(inst.ins, last_mm[kt].ins, sync=False, reason="psum order")
                            last_mm[kt] = inst
                        g4 = mid.tile([4, N], mt, tag="g4", bufs=5, name=f"g4{g}_{l}")
                        nc.vector.tensor_copy(g4[:], gt[0:4, :])
                        S["vt"], S["g4"] = vt, g4
                    # ---------- C / gate-broadcast / cg ----------
                    for g in gs:
                        S = st[g]
                        kc = f"c{g}_{l}"
                        c_ps = pspool.tile([128, 512], f32, tag="cb", bufs=1, name=f"cps{g}_{l}")
                        mm(kc, c_ps[:, 0:N], wt[l][:, C_OFF : C_OFF + 128], S["vt"][:, 0:N], start=True, stop=True)
                        mm(kc, c_ps[:, N : 2 * N], wt[l][:, C_OFF + 128 : C_OFF + 256], S["vt"][:, N : 2 * N], start=False, stop=True)
                        ct = mid.tile([128, 512], mt, tag="ct", bufs=4, name=f"ct{g}_{l}")
                        nc.scalar.activation(ct[:], c_ps[:], AF.Tanh)
                        kb = f"b{g}_{l}"
                        b_ps = pspool.tile([128, 512], f32, tag="cb", bufs=1, name=f"bps{g}_{l}")
                        mm(kb, b_ps[:, 0:N], Pb[:, 0:128], S["g4"][:], start=True, stop=True)
                        mm(kb, b_ps[:, N : 2 * N], Pb[:, 128:256], S["g4"][:], start=False, stop=True)
                        cg = mid.tile([128, 512], mt, tag="cg", bufs=4, name=f"cg{g}_{l}")
                        nc.vector.tensor_mul(cg[:], ct[:], b_ps[:])
                        S["cg"] = cg
                        if l < L - 1:
                            S["xout"] = xpool.tile([128, NCH * N], mt, tag="xl", bufs=8, name=f"xl{g}_{l}")
                        else:
                            S["xout"] = xpool.tile([128, NCH * N], mt, tag="osb", bufs=3, name=f"osb{g}")
                    # ---------- U matmuls + epilogue (quarter granularity) ----------
                    for g in gs:
                        S = st[g]
                        xin, x0t, cg, xout = S["xin"], S["x0t"], S["cg"], S["xout"]
                        for q in range(4):
                            ku = f"u{g}_{l}_{q}"
                            u_ps = pspool.tile([128, 512], f32, tag="u", bufs=3, name=f"ups{g}_{l}_{q}")
                            for mi, mc in enumerate((2 * q, 2 * q + 1)):
                                col = mi * N
                                for kch in range(2):
                                    mm(
                                        ku,
                                        u_ps[:, col : col + N],
                                        wt[l][:, U_OFF + (kch * 8 + mc) * 128 : U_OFF + (kch * 8 + mc + 1) * 128],
                                        cg[:, kch * N : (kch + 1) * N],
                                        start=(mi == 0 and kch == 0),
                                        stop=(kch == 1),
                                    )
                            qs = slice(q * 2 * N, (q + 1) * 2 * N)
                            if bias_nonzero:
                                for mi, mc in enumerate((2 * q, 2 * q + 1)):
                                    col = mi * N
                                    nc.vector.scalar_tensor_tensor(
                                        xout[:, mc * N : (mc + 1) * N],
                                        u_ps[:, col : col + N],
                                        wb_f32[l][:, mc : mc + 1],
                                        x0t[:, mc * N : (mc + 1) * N],
                                        ALU.add,
                                        ALU.mult,
                                    )
                            else:
                                # evacuate PSUM: xout_q = u_ps * x0 (bf16 SBUF out)
                                nc.vector.tensor_mul(xout[:, qs], u_ps[:], x0t[:, qs])
                            # + xl split between Pool and DVE to balance engines
                            eng = nc.gpsimd if q != 3 else nc.vector
                            eng.tensor_add(xout[:, qs], xout[:, qs], xin[:, qs])
                            if q % 2 == 1:
                                hs = slice((q - 1) * 2 * N, (q + 1) * 2 * N)
                                if l < L - 1:
                                    nc.scalar.activation(xout[:, hs], xout[:, hs], AF.Tanh)
                                else:
                                    nc.sync.dma_start(
                                        outT_h[:, q // 2, :, g * N : (g + 1) * N],
                                        xout[:, hs].rearrange("p (c n) -> p c n", c=4),
                                    )
                    for g in gs:
                        st[g]["xin"] = st[g]["xout"]

    # walrus wait-budget legalization on serialization
    orig = nc.to_json_bytes
    nc.to_json_bytes = lambda: _legalize_json_bytes(orig())
    return nc


_CACHE = {}


MODE = "bf16"


def kernel(x, U, V, C, Wg, b):
    import ml_dtypes

    x = np.ascontiguousarray(np.asarray(x, np.float32))
    bias_nonzero = bool(np.any(np.asarray(b) != 0))
    key = ("nc", bias_nonzero, MODE)
    if key not in _CACHE:
        _CACHE[key] = build_nc(bias_nonzero, MODE)
    nc = _CACHE[key]
    mnp = ml_dtypes.bfloat16
    blobs = pack_weights(U, V, C, Wg, b)
    xTfull = np.ascontiguousarray(x.T)  # [D, B]
    wls = {f"wl{l}": np.ascontiguousarray(blobs[f"wl{l}"].astype(mnp)) for l in range(L)}
    wid = np.zeros((128, 388), np.float32)
    wid[:, 0:128] = np.eye(128, dtype=np.float32)
    mcol = np.arange(256)
    wid[0:4, 128:384] = (np.arange(4)[:, None] == (mcol[None, :] // 64)).astype(np.float32)
    wid[0:4, 384:388] = 1.0
    wid = wid.astype(mnp)
    wb = np.stack(
        [blobs[f"wl{l}"][:, B_OFF : B_OFF + NCH] for l in range(L)], axis=1
    ).reshape(128, L * NCH).astype(np.float32)
    in_maps = []
    for m in range(NCORES):
        shard = np.ascontiguousarray(xTfull[:, m * BC : (m + 1) * BC])
        im = {"wid": wid}
        im.update(wls)
        im["xT"] = shard.astype(mnp)
        if bias_nonzero:
            im["wb"] = wb
        in_maps.append(im)
    from concourse import bass2jax

    results = bass2jax.run_bass_via_pjrt(nc, in_maps, n_cores=NCORES)
    out = np.empty((B, D), np.float32)
    for m in range(NCORES):
        out[m * BC : (m + 1) * BC, :] = results[m]["outT"].T.astype(np.float32)
    return out



# revision 5
# speedup vs baseline: 1.0021x; 1.0021x over previous
"""CrossNetMoE forward on 8 Trainium2 NeuronCores (Bass/Tile).

Math (per layer i, E=4 experts, rank R=64, D=1024):
    v = tanh(V_e @ xl)            [B,E,R]
    c = tanh(C_e @ v_e)           [B,E,R]
    g = softmax(Wg_e . xl)        [B,E]
    u = sum_e (g_e * c_e) @ U_e.T + b      (softmax weights sum to 1)
    xl' = tanh(u * x0 + xl)   (last layer: no tanh)

Strategy: pure data-parallel over batch (2048 rows/core), transposed layout
[D, B] on-chip.  The U matmuls (all layers) and the V matmul of layer 0 run
as fp8e4 DoubleRow matmuls (two 128-deep k-tiles contracted per instruction
at 0.5 cycles/row); the moving operands come for free: layer-0 x is sent from
the host in fp8 alongside bf16, and cg (= tanh(c)*gate, all in (-1,1)) is
written in fp8 directly by the DVE multiply.  V matmuls of layers 1-2 stay
bf16 since xl only exists on-chip in bf16.  Softmax: one exp per group (no
ACT accumulator read), DVE segmented reduce for Z.  Epilogue: DVE multiplies
u (f32 PSUM) by x0 for quarters 0-1; quarters 2-3 are DMA-evacuated
PSUM->SBUF so the otherwise-idle Pool engine does those multiplies; the +xl
adds run on DVE at 4x rate (all-bf16, all-SBUF).
"""
import json
import os
import sys

sys.path.insert(0, "/opt/trn_rl_repo")

import numpy as np

L, E, D, R = 3, 4, 1024, 64
B = 16384
NCORES = 8
BC = B // NCORES          # 2048 rows per core
N = 256                   # batch columns per group (matmul moving free dim)
G = BC // N               # 8 groups per core
NCH = D // 128            # 8 d-chunks

# bf16 blob layout (per layer)
# l0:  [C 256 | W 32 | b 8]                      = 296 cols
# l1+: [V 2048 | C 256 | W 32 | b 8]             = 2344 cols
COLS_L0 = 296
COLS_L12 = 2344
# fp8 blob layout: l0: [Vpair 2048 | Upair 2048]; l1+: [Upair 2048]
QCOLS_L0 = 4096
QCOLS_L12 = 2048
# common blob
I_OFF = 0                 # identity 128
P_OFF = 128               # 2 groups x 128 (partitions 0..3)
COLS_C = 388

_EXEMPT = {"Call"}


def _legalize_json_bytes(raw: bytes) -> bytes:
    """Split multi-wait instructions: walrus allows 1 sync-wait per inst."""
    m = json.loads(raw)
    counter = [0]

    def fix_block(block):
        insts = block.get("instructions")
        if insts is not None:
            out = []
            for inst in insts:
                si = inst.get("sync_info")
                if (
                    si
                    and inst.get("opcode") not in _EXEMPT
                    and len(si.get("on_wait") or []) > 1
                ):
                    for w in si["on_wait"][:-1]:
                        counter[0] += 1
                        out.append(
                            {
                                "name": f"I-waitsplit-{counter[0]}",
                                "opcode": "NoOp",
                                "engine": inst["engine"],
                                "ins": [],
                                "outs": [],
                                "debug": 0,
                                "sync_info": {"on_wait": [w], "on_update": []},
                            }
                        )
                    si["on_wait"] = [si["on_wait"][-1]]
                out.append(inst)
            block["instructions"] = out
        for sub in block.get("blocks") or []:
            fix_block(sub)

    for f in m["functions"]:
        for b in f["blocks"]:
            fix_block(b)
    return json.dumps(m).encode()


def _v_pack(V, l):
    """Baseline V chunk pack: [p=d%128, (eg, c, m)] m->(e=2eg+m//64, r=m%64)."""
    p = np.arange(128)
    m = np.arange(128)
    out = np.zeros((128, 2048), np.float32)
    for g in range(2):
        for c in range(NCH):
            out[:, (g * 8 + c) * 128 : (g * 8 + c + 1) * 128] = V[
                l, 2 * g + m[None, :] // 64, m[None, :] % 64, c * 128 + p[:, None]
            ]
    return out


def _u_pack(U, l):
    """Baseline U pack [p=(e,r2)%128 of kchunk, (kc, mc, q)]."""
    p = np.arange(128)
    q = np.arange(128)
    out = np.zeros((128, 2048), np.float32)
    for kc in range(2):
        for mc in range(NCH):
            out[:, (kc * 8 + mc) * 128 : (kc * 8 + mc + 1) * 128] = U[
                l, (kc * 128 + p[:, None]) // 64, mc * 128 + q[None, :],
                (kc * 128 + p[:, None]) % 64,
            ]
    return out


def pack_weights(U, V, C, Wg, b):
    """Pack parameters into SBUF-image blobs (host side)."""
    U, V, C, Wg, b = (np.asarray(a, np.float32) for a in (U, V, C, Wg, b))
    blobs = {}
    p = np.arange(128)
    for l in range(L):
        cols = COLS_L0 if l == 0 else COLS_L12
        voff = 0 if l == 0 else 2048
        blob = np.zeros((128, cols), np.float32)
        if l > 0:
            blob[:, 0:2048] = _v_pack(V, l)
        # Cw: blockdiag pairs: [p=(el',r1), (g,j=(el,r2))]
        for g in range(2):
            j = np.arange(128)
            el_p = p[:, None] // 64
            el_j = j[None, :] // 64
            val = C[l, 2 * g + el_j, j[None, :] % 64, p[:, None] % 64]
            blob[:, voff + g * 128 : voff + (g + 1) * 128] = np.where(
                el_p == el_j, val, 0.0
            )
        # Ww: [p=d%128, (c,e)]
        for c in range(NCH):
            blob[:, voff + 256 + c * 4 : voff + 256 + (c + 1) * 4] = Wg[
                l, :, c * 128 + p
            ]
        for c in range(NCH):
            blob[:, voff + 288 + c] = b[l, c * 128 + p]
        blobs[f"wl{l}"] = blob
        # fp8 blob: U DoubleRow pairs: for mc: [ktileA=Upack(kc0,mc) | ktileB=Upack(kc1,mc)]
        up = _u_pack(U, l)
        qcols = QCOLS_L0 if l == 0 else QCOLS_L12
        qb = np.zeros((128, qcols), np.float32)
        uoff = 2048 if l == 0 else 0
        for mc in range(NCH):
            qb[:, uoff + mc * 256 : uoff + mc * 256 + 128] = up[:, mc * 128 : (mc + 1) * 128]
            qb[:, uoff + mc * 256 + 128 : uoff + mc * 256 + 256] = up[
                :, (8 + mc) * 128 : (9 + mc) * 128
            ]
        if l == 0:
            vp = _v_pack(V, 0)
            for g in range(2):
                for cp in range(4):
                    base = (g * 4 + cp) * 256
                    qb[:, base : base + 128] = vp[
                        :, (g * 8 + 2 * cp) * 128 : (g * 8 + 2 * cp + 1) * 128
                    ]
                    qb[:, base + 128 : base + 256] = vp[
                        :, (g * 8 + 2 * cp + 1) * 128 : (g * 8 + 2 * cp + 2) * 128
                    ]
        blobs[f"wq{l}"] = qb
    wc = np.zeros((128, COLS_C), np.float32)
    wc[:, 0:128] = np.eye(128, dtype=np.float32)
    for g in range(2):
        m = np.arange(128)
        wc[0:4, P_OFF + g * 128 : P_OFF + (g + 1) * 128] = (
            np.arange(4)[:, None] == (2 * g + m[None, :] // 64)
        ).astype(np.float32)
    blobs["wc"] = wc
    return blobs


def build_nc(bias_nonzero=False, mode="bf16"):
    import concourse.bass as bass
    import concourse.tile as tile
    from concourse import mybir
    from concourse.tile import add_dep_helper

    f32 = mybir.dt.float32
    AF = mybir.ActivationFunctionType
    ALU = mybir.AluOpType
    bf16 = mybir.dt.bfloat16
    fp8 = mybir.dt.float8e4
    DR = mybir.MatmulPerfMode.DoubleRow
    mt = bf16

    nc = bass.Bass()
    xT = nc.dram_tensor("xT", [D, BC], mt, kind="ExternalInput")
    xQ = nc.dram_tensor("xQ", [D, BC], fp8, kind="ExternalInput")
    wl = [
        nc.dram_tensor(f"wl{l}", [128, COLS_L0 if l == 0 else COLS_L12], mt,
                       kind="ExternalInput")
        for l in range(L)
    ]
    wq = [
        nc.dram_tensor(f"wq{l}", [128, QCOLS_L0 if l == 0 else QCOLS_L12], fp8,
                       kind="ExternalInput")
        for l in range(L)
    ]
    wbd = nc.dram_tensor("wb", [128, L * NCH], f32, kind="ExternalInput") if bias_nonzero else None
    wid = nc.dram_tensor("wid", [128, COLS_C], mt, kind="ExternalInput")
    outT = nc.dram_tensor("outT", [D, BC], mt, kind="ExternalOutput")

    # chain matmuls that share a psum tile so scheduler keeps program order
    last_mm = {}

    def mm(key, out, lhsT, rhs, start, stop, perf_mode=None):
        inst = nc.tensor.matmul(
            out, lhsT, rhs, start=start, stop=stop, skip_group_check=True,
            perf_mode=perf_mode,
        )
        if key in last_mm:
            add_dep_helper(inst.ins, last_mm[key].ins, sync=False, reason="psum order")
        last_mm[key] = inst
        return inst

    with tile.TileContext(nc) as tc:
        with (
            tc.tile_pool(name="wpool", bufs=1) as wpool,
            tc.tile_pool(name="xpool", bufs=1) as xpool,
            tc.tile_pool(name="mid", bufs=1) as mid,
            tc.tile_pool(name="pspool", bufs=1, space="PSUM") as pspool,
            nc.allow_low_precision(reason="bf16/fp8 matmul pipeline (intentional)"),
        ):
            wt = []
            wqt = []
            for l in range(L):
                wt.append(wpool.tile([128, COLS_L0 if l == 0 else COLS_L12], mt, name=f"wt{l}"))
                wqt.append(wpool.tile([128, QCOLS_L0 if l == 0 else QCOLS_L12], fp8, name=f"wq{l}"))
            wb_f32 = None
            identm = wpool.tile([128, COLS_C], mt, name="identm")
            Pb = identm[0:4, 128:384]

            def VOFF(l):
                return 0 if l == 0 else 0  # V cols start at 0 for l>=1
            def COFF(l):
                return 0 if l == 0 else 2048
            def WOFF(l):
                return 256 if l == 0 else 2304
            def BOFF(l):
                return 288 if l == 0 else 2336

            xT_v = xT.rearrange("(c p) b -> p c b", p=128)
            xQ_v = xQ.rearrange("(c p) b -> p c b", p=128)
            outT_h = outT.rearrange("(h c p) b -> p h c b", p=128, h=2)

            ROT = 4  # groups processed in rotation
            for sup in range(G // ROT):
                gs = [ROT * sup + i for i in range(ROT)]
                st = {}
                if sup == 0:
                    # startup: layer-0 fp8 weights + first x groups first so PE can start
                    nc.sync.dma_start(wqt[0][:], wq[0][:])
                    nc.scalar.dma_start(identm[:], wid[:])
                    for i, g in enumerate(gs):
                        x0t = xpool.tile([128, NCH * N], mt, tag="x0", bufs=8, name=f"x0_{g}")
                        x0q = xpool.tile([128, NCH * N], fp8, tag="xq", bufs=4, name=f"xq_{g}")
                        (nc.sync if i == 0 else nc.scalar).dma_start(
                            x0q[:], xQ_v[:, :, g * N : (g + 1) * N]
                        )
                        (nc.sync if i == 0 else nc.scalar).dma_start(
                            x0t[:], xT_v[:, :, g * N : (g + 1) * N]
                        )
                        if i == 0:
                            nc.sync.dma_start(wt[0][:], wl[0][:])
                        st[g] = dict(x0t=x0t, x0q=x0q, xin=x0t)
                    if bias_nonzero:
                        wbt = wpool.tile([128, L * NCH], f32)
                        nc.sync.dma_start(wbt[:], wbd[:])
                        wb_f32 = [wbt[:, l * NCH : (l + 1) * NCH] for l in range(L)]
                    for l_ in range(1, L):
                        nc.scalar.dma_start(wt[l_][:], wl[l_][:])
                        nc.sync.dma_start(wqt[l_][:], wq[l_][:])
                else:
                    for i, g in enumerate(gs):
                        x0t = xpool.tile([128, NCH * N], mt, tag="x0", bufs=8, name=f"x0_{g}")
                        x0q = xpool.tile([128, NCH * N], fp8, tag="xq", bufs=4, name=f"xq_{g}")
                        nc.sync.dma_start(x0t[:], xT_v[:, :, g * N : (g + 1) * N])
                        nc.scalar.dma_start(x0q[:], xQ_v[:, :, g * N : (g + 1) * N])
                        st[g] = dict(x0t=x0t, x0q=x0q, xin=x0t)

                for l in range(L):
                    # ---------- P1 + per-group softmax chain ----------
                    for gi, g in enumerate(gs):
                        S = st[g]
                        xin = S["xin"]
                        kv = f"v{g}_{l}"
                        v_ps = pspool.tile([128, 512], f32, tag="v", bufs=2, name=f"vps{g}_{l}")
                        if l == 0:
                            xq = S["x0q"]
                            for eg in range(2):
                                for cp in range(4):
                                    mm(
                                        kv,
                                        v_ps[:, eg * N : (eg + 1) * N],
                                        wqt[0][:, (eg * 4 + cp) * 256 : (eg * 4 + cp + 1) * 256]
                                        .rearrange("p (two m) -> p two m", two=2),
                                        xq[:, cp * 512 : (cp + 1) * 512]
                                        .rearrange("p (two n) -> p two n", two=2),
                                        start=(cp == 0),
                                        stop=(cp == 3),
                                        perf_mode=DR,
                                    )
                        else:
                            for c in range(NCH):
                                rhs = xin[:, c * N : (c + 1) * N]
                                mm(kv, v_ps[:, 0:N], wt[l][:, c * 128 : (c + 1) * 128], rhs, start=(c == 0), stop=False)
                                mm(kv, v_ps[:, N : 2 * N], wt[l][:, (8 + c) * 128 : (9 + c) * 128], rhs, start=False, stop=(c == NCH - 1))
                        # gate logits in transposed [batch_p, expert] layout
                        ks = f"sb{g}_{l}"
                        sbt = pspool.tile([128, 8], f32, tag="sbt", bufs=1, name=f"sbt{g}_{l}")
                        for h in range(2):
                            for c in range(NCH):
                                mm(
                                    ks,
                                    sbt[:, h * 4 : (h + 1) * 4],
                                    xin[:, c * N + h * 128 : c * N + h * 128 + 128],
                                    wt[l][:, WOFF(l) + c * 4 : WOFF(l) + (c + 1) * 4],
                                    start=(c == 0),
                                    stop=(c == NCH - 1),
                                )
                        # chain: tanh(v); softmax over the 4-col free dim
                        vt = mid.tile([128, 512], mt, tag="vt", bufs=5, name=f"vt{g}_{l}")
                        nc.scalar.activation(vt[:], v_ps[:], AF.Tanh)
                        ebt = mid.tile([128, 8], f32, tag="ebt", bufs=5, name=f"ebt{g}_{l}")
                        nc.scalar.activation(ebt[:], sbt[:], AF.Exp)
                        zsum = mid.tile([128, 2], f32, tag="zs", bufs=5, name=f"zs{g}_{l}")
                        nc.vector.tensor_reduce(
                            zsum[:],
                            ebt[:].rearrange("p (g x) -> p g x", g=2),
                            mybir.AxisListType.X,
                            ALU.add,
                        )
                        rz = mid.tile([128, 2], f32, tag="rz", bufs=5, name=f"rz{g}_{l}")
                        nc.vector.reciprocal(rz[:], zsum[:])
                        gbt = mid.tile([128, 8], mt, tag="gbt", bufs=5, name=f"gbt{g}_{l}")
                        for h in range(2):
                            nc.vector.tensor_scalar_mul(
                                gbt[:, h * 4 : (h + 1) * 4],
                                ebt[:, h * 4 : (h + 1) * 4],
                                rz[:, h : h + 1],
                            )
                        # transpose gates back to [expert, batch] (bf16 PSUM out)
                        kt = f"gt{g}_{l}"
                        gt = pspool.tile([4, 256], mt, tag="gt", bufs=1, name=f"gt{g}_{l}")
                        for h in range(2):
                            inst = nc.tensor.transpose(
                                gt[0:4, h * 128 : (h + 1) * 128],
                                gbt[:, h * 4 : (h + 1) * 4],
                                identm[:, 0:128],
                            )
                            if kt in last_mm:
                                add_dep_helper(inst.ins, last_mm[kt].ins, sync=False, reason="psum order")
                            last_mm[kt] = inst
                        g4 = mid.tile([4, N], mt, tag="g4", bufs=5, name=f"g4{g}_{l}")
                        nc.vector.tensor_copy(g4[:], gt[0:4, :])
                        S["vt"], S["g4"] = vt, g4
                    # ---------- C / gate-broadcast / cg ----------
                    for g in gs:
                        S = st[g]
                        kc = f"c{g}_{l}"
                        c_ps = pspool.tile([128, 512], f32, tag="cb", bufs=1, name=f"cps{g}_{l}")
                        mm(kc, c_ps[:, 0:N], wt[l][:, COFF(l) : COFF(l) + 128], S["vt"][:, 0:N], start=True, stop=True)
                        mm(kc, c_ps[:, N : 2 * N], wt[l][:, COFF(l) + 128 : COFF(l) + 256], S["vt"][:, N : 2 * N], start=False, stop=True)
                        ct = mid.tile([128, 512], mt, tag="ct", bufs=4, name=f"ct{g}_{l}")
                        nc.scalar.activation(ct[:], c_ps[:], AF.Tanh)
                        kb = f"b{g}_{l}"
                        b_ps = pspool.tile([128, 512], f32, tag="cb", bufs=1, name=f"bps{g}_{l}")
                        mm(kb, b_ps[:, 0:N], Pb[:, 0:128], S["g4"][:], start=True, stop=True)
                        mm(kb, b_ps[:, N : 2 * N], Pb[:, 128:256], S["g4"][:], start=False, stop=True)
                        cg = mid.tile([128, 512], fp8, tag="cg", bufs=4, name=f"cg{g}_{l}")
                        nc.vector.tensor_mul(cg[:], ct[:], b_ps[:])
                        S["cg"] = cg
                        if l < L - 1:
                            S["xout"] = xpool.tile([128, NCH * N], mt, tag="xl", bufs=8, name=f"xl{g}_{l}")
                        else:
                            S["xout"] = xpool.tile([128, NCH * N], mt, tag="osb", bufs=3, name=f"osb{g}")
                    # ---------- U matmuls (fp8 DoubleRow) + epilogue ----------
                    for g in gs:
                        S = st[g]
                        xin, x0t, cg, xout = S["xin"], S["x0t"], S["cg"], S["xout"]
                        cg2 = cg[:].rearrange("p (two n) -> p two n", two=2)
                        for q in range(4):
                            ku = f"u{g}_{l}_{q}"
                            u_ps = pspool.tile([128, 512], f32, tag="u", bufs=3, name=f"ups{g}_{l}_{q}")
                            for mi, mc in enumerate((2 * q, 2 * q + 1)):
                                mm(
                                    ku,
                                    u_ps[:, mi * N : (mi + 1) * N],
                                    wqt[l][
                                        :,
                                        (2048 if l == 0 else 0) + mc * 256 : (2048 if l == 0 else 0) + (mc + 1) * 256,
                                    ].rearrange("p (two m) -> p two m", two=2),
                                    cg2,
                                    start=(mi == 0),
                                    stop=(mi == 1),
                                    perf_mode=DR,
                                )
                            qs = slice(q * 2 * N, (q + 1) * 2 * N)
                            if bias_nonzero:
                                for mi, mc in enumerate((2 * q, 2 * q + 1)):
                                    col = mi * N
                                    nc.vector.scalar_tensor_tensor(
                                        xout[:, mc * N : (mc + 1) * N],
                                        u_ps[:, col : col + N],
                                        wb_f32[l][:, mc : mc + 1],
                                        x0t[:, mc * N : (mc + 1) * N],
                                        ALU.add,
                                        ALU.mult,
                                    )
                            elif q != 3:
                                # evacuate PSUM: xout_q = u_ps * x0 on DVE
                                nc.vector.tensor_mul(xout[:, qs], u_ps[:], x0t[:, qs])
                            else:
                                # ACT evacuates one quarter; Pool multiplies (SBUF-only)
                                usb = mid.tile([128, 512], f32, tag="usb", bufs=3, name=f"usb{g}_{l}_{q}")
                                nc.scalar.activation(usb[:], u_ps[:], AF.Copy)
                                nc.gpsimd.tensor_mul(xout[:, qs], usb[:], x0t[:, qs])
                            if q % 2 == 1:
                                hs = slice((q - 1) * 2 * N, (q + 1) * 2 * N)
                                # +xl at DVE 4x rate (all bf16, all SBUF)
                                nc.vector.tensor_add(xout[:, hs], xout[:, hs], xin[:, hs])
                                if l < L - 1:
                                    nc.scalar.activation(xout[:, hs], xout[:, hs], AF.Tanh)
                                else:
                                    nc.sync.dma_start(
                                        outT_h[:, q // 2, :, g * N : (g + 1) * N],
                                        xout[:, hs].rearrange("p (c n) -> p c n", c=4),
                                    )
                    for g in gs:
                        st[g]["xin"] = st[g]["xout"]

    # walrus wait-budget legalization on serialization
    orig = nc.to_json_bytes
    nc.to_json_bytes = lambda: _legalize_json_bytes(orig())
    return nc


_CACHE = {}


MODE = "bf16"


def kernel(x, U, V, C, Wg, b):
    import ml_dtypes

    x = np.ascontiguousarray(np.asarray(x, np.float32))
    bias_nonzero = bool(np.any(np.asarray(b) != 0))
    key = ("nc", bias_nonzero, MODE)
    if key not in _CACHE:
        _CACHE[key] = build_nc(bias_nonzero, MODE)
    nc = _CACHE[key]
    mnp = ml_dtypes.bfloat16
    f8np = ml_dtypes.float8_e4m3
    blobs = pack_weights(U, V, C, Wg, b)
    xTfull = np.ascontiguousarray(x.T)  # [D, B]
    wls = {f"wl{l}": np.ascontiguousarray(blobs[f"wl{l}"].astype(mnp)) for l in range(L)}
    wqs = {f"wq{l}": np.ascontiguousarray(blobs[f"wq{l}"].astype(f8np)) for l in range(L)}
    wid = np.zeros((128, COLS_C), np.float32)
    wid[:, 0:128] = np.eye(128, dtype=np.float32)
    mcol = np.arange(256)
    wid[0:4, 128:384] = (np.arange(4)[:, None] == (mcol[None, :] // 64)).astype(np.float32)
    wid = wid.astype(mnp)
    wb = np.stack(
        [blobs[f"wl{l}"][:, (288 if l == 0 else 2336) : (288 if l == 0 else 2336) + NCH] for l in range(L)], axis=1
    ).reshape(128, L * NCH).astype(np.float32)
    in_maps = []
    for m in range(NCORES):
        shard = np.ascontiguousarray(xTfull[:, m * BC : (m + 1) * BC])
        im = {"wid": wid}
        im.update(wls)
        im.update(wqs)
        im["xT"] = shard.astype(mnp)
        im["xQ"] = shard.astype(f8np)
        if bias_nonzero:
            im["wb"] = wb
        in_maps.append(im)
    from concourse import bass2jax

    results = bass2jax.run_bass_via_pjrt(nc, in_maps, n_cores=NCORES)
    out = np.empty((B, D), np.float32)
    for m in range(NCORES):
        out[m * BC : (m + 1) * BC, :] = results[m]["outT"].T.astype(np.float32)
    return out


# revision 12
# speedup vs baseline: 1.0060x; 1.0039x over previous
"""CrossNetMoE forward on 8 Trainium2 NeuronCores (Bass/Tile).

Math (per layer i, E=4 experts, rank R=64, D=1024):
    v = tanh(V_e @ xl)            [B,E,R]
    c = tanh(C_e @ v_e)           [B,E,R]
    g = softmax(Wg_e . xl)        [B,E]
    u = sum_e (g_e * c_e) @ U_e.T + b      (softmax weights sum to 1)
    xl' = tanh(u * x0 + xl)   (last layer: no tanh)

Strategy: pure data-parallel over batch (2048 rows/core), transposed layout
[D, B] on-chip.  The U matmuls (all layers) and the V matmul of layer 0 run
as fp8e4 DoubleRow matmuls (two 128-deep k-tiles contracted per instruction
at 0.5 cycles/row); the moving operands come for free: layer-0 x is sent from
the host in fp8 alongside bf16, and cg (= tanh(c)*gate, all in (-1,1)) is
written in fp8 directly by the DVE multiply.  V matmuls of layers 1-2 stay
bf16 since xl only exists on-chip in bf16.  Softmax: one exp per group (no
ACT accumulator read), DVE segmented reduce for Z.  Epilogue: DVE multiplies
u (f32 PSUM) by x0 for quarters 0-1; quarters 2-3 are DMA-evacuated
PSUM->SBUF so the otherwise-idle Pool engine does those multiplies; the +xl
adds run on DVE at 4x rate (all-bf16, all-SBUF).
"""
import json
import os
import sys

sys.path.insert(0, "/opt/trn_rl_repo")

import numpy as np

L, E, D, R = 3, 4, 1024, 64
B = 16384
NCORES = 8
BC = B // NCORES          # 2048 rows per core
N = 256                   # batch columns per group (matmul moving free dim)
G = BC // N               # 8 groups per core
NCH = D // 128            # 8 d-chunks

# bf16 blob layout (per layer)
# l0:  [C 256 | W 32 | b 8]                      = 296 cols
# l1+: [V 2048 | C 256 | W 32 | b 8]             = 2344 cols
COLS_L0 = 296
COLS_L12 = 2344
# fp8 blob layout: l0: [Vpair 2048 | Upair 2048]; l1+: [Upair 2048]
QCOLS_L0 = 4096
QCOLS_L12 = 2048
# common blob
I_OFF = 0                 # identity 128
P_OFF = 128               # 2 groups x 128 (partitions 0..3)
COLS_C = 388

_EXEMPT = {"Call"}


def _legalize_json_bytes(raw: bytes) -> bytes:
    """Split multi-wait instructions: walrus allows 1 sync-wait per inst."""
    m = json.loads(raw)
    counter = [0]

    def fix_block(block):
        insts = block.get("instructions")
        if insts is not None:
            out = []
            for inst in insts:
                si = inst.get("sync_info")
                if (
                    si
                    and inst.get("opcode") not in _EXEMPT
                    and len(si.get("on_wait") or []) > 1
                ):
                    for w in si["on_wait"][:-1]:
                        counter[0] += 1
                        out.append(
                            {
                                "name": f"I-waitsplit-{counter[0]}",
                                "opcode": "NoOp",
                                "engine": inst["engine"],
                                "ins": [],
                                "outs": [],
                                "debug": 0,
                                "sync_info": {"on_wait": [w], "on_update": []},
                            }
                        )
                    si["on_wait"] = [si["on_wait"][-1]]
                out.append(inst)
            block["instructions"] = out
        for sub in block.get("blocks") or []:
            fix_block(sub)

    for f in m["functions"]:
        for b in f["blocks"]:
            fix_block(b)
    return json.dumps(m).encode()


def _v_pack(V, l):
    """Baseline V chunk pack: [p=d%128, (eg, c, m)] m->(e=2eg+m//64, r=m%64)."""
    p = np.arange(128)
    m = np.arange(128)
    out = np.zeros((128, 2048), np.float32)
    for g in range(2):
        for c in range(NCH):
            out[:, (g * 8 + c) * 128 : (g * 8 + c + 1) * 128] = V[
                l, 2 * g + m[None, :] // 64, m[None, :] % 64, c * 128 + p[:, None]
            ]
    return out


def _u_pack(U, l):
    """Baseline U pack [p=(e,r2)%128 of kchunk, (kc, mc, q)]."""
    p = np.arange(128)
    q = np.arange(128)
    out = np.zeros((128, 2048), np.float32)
    for kc in range(2):
        for mc in range(NCH):
            out[:, (kc * 8 + mc) * 128 : (kc * 8 + mc + 1) * 128] = U[
                l, (kc * 128 + p[:, None]) // 64, mc * 128 + q[None, :],
                (kc * 128 + p[:, None]) % 64,
            ]
    return out


def pack_weights(U, V, C, Wg, b):
    """Pack parameters into SBUF-image blobs (host side)."""
    U, V, C, Wg, b = (np.asarray(a, np.float32) for a in (U, V, C, Wg, b))
    blobs = {}
    p = np.arange(128)
    for l in range(L):
        cols = COLS_L0 if l == 0 else COLS_L12
        voff = 0 if l == 0 else 2048
        blob = np.zeros((128, cols), np.float32)
        if l > 0:
            blob[:, 0:2048] = _v_pack(V, l)
        # Cw: blockdiag pairs: [p=(el',r1), (g,j=(el,r2))]
        for g in range(2):
            j = np.arange(128)
            el_p = p[:, None] // 64
            el_j = j[None, :] // 64
            val = C[l, 2 * g + el_j, j[None, :] % 64, p[:, None] % 64]
            blob[:, voff + g * 128 : voff + (g + 1) * 128] = np.where(
                el_p == el_j, val, 0.0
            )
        # Ww: [p=d%128, (c,e)]
        for c in range(NCH):
            blob[:, voff + 256 + c * 4 : voff + 256 + (c + 1) * 4] = Wg[
                l, :, c * 128 + p
            ]
        for c in range(NCH):
            blob[:, voff + 288 + c] = b[l, c * 128 + p]
        blobs[f"wl{l}"] = blob
        # fp8 blob: U DoubleRow pairs: for mc: [ktileA=Upack(kc0,mc) | ktileB=Upack(kc1,mc)]
        up = _u_pack(U, l)
        qcols = QCOLS_L0 if l == 0 else QCOLS_L12
        qb = np.zeros((128, qcols), np.float32)
        uoff = 2048 if l == 0 else 0
        for mc in range(NCH):
            qb[:, uoff + mc * 256 : uoff + mc * 256 + 128] = up[:, mc * 128 : (mc + 1) * 128]
            qb[:, uoff + mc * 256 + 128 : uoff + mc * 256 + 256] = up[
                :, (8 + mc) * 128 : (9 + mc) * 128
            ]
        if l == 0:
            vp = _v_pack(V, 0)
            for g in range(2):
                for cp in range(4):
                    base = (g * 4 + cp) * 256
                    qb[:, base : base + 128] = vp[
                        :, (g * 8 + 2 * cp) * 128 : (g * 8 + 2 * cp + 1) * 128
                    ]
                    qb[:, base + 128 : base + 256] = vp[
                        :, (g * 8 + 2 * cp + 1) * 128 : (g * 8 + 2 * cp + 2) * 128
                    ]
        blobs[f"wq{l}"] = qb
    wc = np.zeros((128, COLS_C), np.float32)
    wc[:, 0:128] = np.eye(128, dtype=np.float32)
    for g in range(2):
        m = np.arange(128)
        wc[0:4, P_OFF + g * 128 : P_OFF + (g + 1) * 128] = (
            np.arange(4)[:, None] == (2 * g + m[None, :] // 64)
        ).astype(np.float32)
    blobs["wc"] = wc
    return blobs


def build_nc(bias_nonzero=False, mode="bf16"):
    import concourse.bass as bass
    import concourse.tile as tile
    from concourse import mybir
    from concourse.tile import add_dep_helper

    f32 = mybir.dt.float32
    AF = mybir.ActivationFunctionType
    ALU = mybir.AluOpType
    bf16 = mybir.dt.bfloat16
    fp8 = mybir.dt.float8e4
    DR = mybir.MatmulPerfMode.DoubleRow
    mt = bf16

    nc = bass.Bass()
    xT = nc.dram_tensor("xT", [D, BC], mt, kind="ExternalInput")
    xQ = nc.dram_tensor("xQ", [D, BC], fp8, kind="ExternalInput")
    wl = [
        nc.dram_tensor(f"wl{l}", [128, COLS_L0 if l == 0 else COLS_L12], mt,
                       kind="ExternalInput")
        for l in range(L)
    ]
    wq = [
        nc.dram_tensor(f"wq{l}", [128, QCOLS_L0 if l == 0 else QCOLS_L12], fp8,
                       kind="ExternalInput")
        for l in range(L)
    ]
    wbd = nc.dram_tensor("wb", [128, L * NCH], f32, kind="ExternalInput") if bias_nonzero else None
    wid = nc.dram_tensor("wid", [128, COLS_C], mt, kind="ExternalInput")
    outT = nc.dram_tensor("outT", [D, BC], mt, kind="ExternalOutput")

    # chain matmuls that share a psum tile so scheduler keeps program order
    last_mm = {}

    def mm(key, out, lhsT, rhs, start, stop, perf_mode=None):
        inst = nc.tensor.matmul(
            out, lhsT, rhs, start=start, stop=stop, skip_group_check=True,
            perf_mode=perf_mode,
        )
        if key in last_mm:
            add_dep_helper(inst.ins, last_mm[key].ins, sync=False, reason="psum order")
        last_mm[key] = inst
        return inst

    with tile.TileContext(nc) as tc:
        with (
            tc.tile_pool(name="wpool", bufs=1) as wpool,
            tc.tile_pool(name="xpool", bufs=1) as xpool,
            tc.tile_pool(name="mid", bufs=1) as mid,
            tc.tile_pool(name="pspool", bufs=1, space="PSUM") as pspool,
            nc.allow_low_precision(reason="bf16/fp8 matmul pipeline (intentional)"),
        ):
            wt = []
            wqt = []
            for l in range(L):
                wt.append(wpool.tile([128, COLS_L0 if l == 0 else COLS_L12], mt, name=f"wt{l}"))
                wqt.append(wpool.tile([128, QCOLS_L0 if l == 0 else QCOLS_L12], fp8, name=f"wq{l}"))
            wb_f32 = None
            identm = wpool.tile([128, COLS_C], mt, name="identm")
            Pb = identm[0:4, 128:384]

            def VOFF(l):
                return 0 if l == 0 else 0  # V cols start at 0 for l>=1
            def COFF(l):
                return 0 if l == 0 else 2048
            def WOFF(l):
                return 256 if l == 0 else 2304
            def BOFF(l):
                return 288 if l == 0 else 2336

            xT_v = xT.rearrange("(c p) b -> p c b", p=128)
            xQ_v = xQ.rearrange("(c p) b -> p c b", p=128)
            outT_h = outT.rearrange("(h c p) b -> p h c b", p=128, h=2)

            ROT = int(os.environ.get('K_ROT', '4'))  # groups processed in rotation
            for sup in range(G // ROT):
                gs = [ROT * sup + i for i in range(ROT)]
                st = {}
                if sup == 0:
                    # startup: fan the critical tiles across 4 DMA queues so
                    # layer-0 compute starts as early as possible
                    qs_ = [nc.sync, nc.gpsimd, nc.scalar]
                    # group-0 fp8 x + V0 fp8 weights first (v matmul l0)
                    x0t = xpool.tile([128, NCH * N], mt, tag="x0", bufs=8, name="x0_0")
                    x0q = xpool.tile([128, NCH * N], fp8, tag="xq", bufs=int(os.environ.get("K_ROT", "4")), name="xq_0")
                    nc.sync.dma_start(x0q[:], xQ_v[:, :, 0:N])
                    nc.gpsimd.dma_start(wqt[0][:, 0:2048], wq[0][:, 0:2048])
                    nc.scalar.dma_start(x0t[:], xT_v[:, :, 0:N])
                    nc.gpsimd.dma_start(wt[0][:], wl[0][:])
                    nc.gpsimd.dma_start(identm[:], wid[:])
                    st[gs[0]] = dict(x0t=x0t, x0q=x0q, xin=x0t)
                    for i, g in enumerate(gs[1:]):
                        x0t = xpool.tile([128, NCH * N], mt, tag="x0", bufs=8, name=f"x0_{g}")
                        x0q = xpool.tile([128, NCH * N], fp8, tag="xq", bufs=int(os.environ.get("K_ROT", "4")), name=f"xq_{g}")
                        qs_[i % 3].dma_start(x0q[:], xQ_v[:, :, g * N : (g + 1) * N])
                        qs_[(i + 1) % 3].dma_start(x0t[:], xT_v[:, :, g * N : (g + 1) * N])
                        st[g] = dict(x0t=x0t, x0q=x0q, xin=x0t)
                    # U0 fp8 + remaining layers, round-robin across queues
                    nc.sync.dma_start(wqt[0][:, 2048:4096], wq[0][:, 2048:4096])
                    if bias_nonzero:
                        wbt = wpool.tile([128, L * NCH], f32)
                        nc.sync.dma_start(wbt[:], wbd[:])
                        wb_f32 = [wbt[:, l * NCH : (l + 1) * NCH] for l in range(L)]
                    for l_ in range(1, L):
                        qs_[(l_ * 2) % 3].dma_start(wt[l_][:, 0:1172], wl[l_][:, 0:1172])
                        qs_[(l_ * 2 + 1) % 3].dma_start(wt[l_][:, 1172:COLS_L12], wl[l_][:, 1172:COLS_L12])
                        qs_[(l_ * 2 + 2) % 3].dma_start(wqt[l_][:], wq[l_][:])
                else:
                    for i, g in enumerate(gs):
                        x0t = xpool.tile([128, NCH * N], mt, tag="x0", bufs=8, name=f"x0_{g}")
                        x0q = xpool.tile([128, NCH * N], fp8, tag="xq", bufs=int(os.environ.get("K_ROT", "4")), name=f"xq_{g}")
                        nc.sync.dma_start(x0t[:], xT_v[:, :, g * N : (g + 1) * N])
                        nc.scalar.dma_start(x0q[:], xQ_v[:, :, g * N : (g + 1) * N])
                        st[g] = dict(x0t=x0t, x0q=x0q, xin=x0t)

                for l in range(L):
                    # ---------- P1 + per-group softmax chain ----------
                    for gi, g in enumerate(gs):
                        S = st[g]
                        xin = S["xin"]
                        kv = f"v{g}_{l}"
                        v_ps = pspool.tile([128, 512], f32, tag="v", bufs=int(os.environ.get("K_V", "1")), name=f"vps{g}_{l}")
                        if l == 0:
                            xq = S["x0q"]
                            for eg in range(2):
                                for cp in range(4):
                                    mm(
                                        kv,
                                        v_ps[:, eg * N : (eg + 1) * N],
                                        wqt[0][:, (eg * 4 + cp) * 256 : (eg * 4 + cp + 1) * 256]
                                        .rearrange("p (two m) -> p two m", two=2),
                                        xq[:, cp * 512 : (cp + 1) * 512]
                                        .rearrange("p (two n) -> p two n", two=2),
                                        start=(cp == 0),
                                        stop=(cp == 3),
                                        perf_mode=DR,
                                    )
                        else:
                            for c in range(NCH):
                                rhs = xin[:, c * N : (c + 1) * N]
                                mm(kv, v_ps[:, 0:N], wt[l][:, c * 128 : (c + 1) * 128], rhs, start=(c == 0), stop=False)
                                mm(kv, v_ps[:, N : 2 * N], wt[l][:, (8 + c) * 128 : (9 + c) * 128], rhs, start=False, stop=(c == NCH - 1))
                        # gate logits in transposed [batch_p, expert] layout
                        ks = f"sb{g}_{l}"
                        sbt = pspool.tile([128, 8], f32, tag="sbt", bufs=int(os.environ.get("K_SBT", "1")), name=f"sbt{g}_{l}")
                        for h in range(2):
                            for c in range(NCH):
                                mm(
                                    ks,
                                    sbt[:, h * 4 : (h + 1) * 4],
                                    xin[:, c * N + h * 128 : c * N + h * 128 + 128],
                                    wt[l][:, WOFF(l) + c * 4 : WOFF(l) + (c + 1) * 4],
                                    start=(c == 0),
                                    stop=(c == NCH - 1),
                                )
                        # chain: tanh(v); softmax over the 4-col free dim
                        vt = mid.tile([128, 512], mt, tag="vt", bufs=5, name=f"vt{g}_{l}")
                        nc.scalar.activation(vt[:], v_ps[:], AF.Tanh)
                        ebt = mid.tile([128, 8], f32, tag="ebt", bufs=5, name=f"ebt{g}_{l}")
                        nc.scalar.activation(ebt[:], sbt[:], AF.Exp)
                        zsum = mid.tile([128, 2], f32, tag="zs", bufs=5, name=f"zs{g}_{l}")
                        nc.vector.tensor_reduce(
                            zsum[:],
                            ebt[:].rearrange("p (g x) -> p g x", g=2),
                            mybir.AxisListType.X,
                            ALU.add,
                        )
                        rz = mid.tile([128, 2], f32, tag="rz", bufs=5, name=f"rz{g}_{l}")
                        nc.vector.reciprocal(rz[:], zsum[:])
                        gbt = mid.tile([128, 8], mt, tag="gbt", bufs=5, name=f"gbt{g}_{l}")
                        for h in range(2):
                            nc.vector.tensor_scalar_mul(
                                gbt[:, h * 4 : (h + 1) * 4],
                                ebt[:, h * 4 : (h + 1) * 4],
                                rz[:, h : h + 1],
                            )
                        # transpose gates back to [expert, batch] (bf16 PSUM out)
                        kt = f"gt{g}_{l}"
                        gt = pspool.tile([4, 256], mt, tag="gt", bufs=int(os.environ.get("K_GT", "1")), name=f"gt{g}_{l}")
                        for h in range(2):
                            inst = nc.tensor.transpose(
                                gt[0:4, h * 128 : (h + 1) * 128],
                                gbt[:, h * 4 : (h + 1) * 4],
                                identm[:, 0:128],
                            )
                            if kt in last_mm:
                                add_dep_helper(inst.ins, last_mm[kt].ins, sync=False, reason="psum order")
                            last_mm[kt] = inst
                        g4 = mid.tile([4, N], mt, tag="g4", bufs=5, name=f"g4{g}_{l}")
                        nc.vector.tensor_copy(g4[:], gt[0:4, :])
                        S["vt"], S["g4"] = vt, g4
                    # ---------- C / gate-broadcast / cg ----------
                    for g in gs:
                        S = st[g]
                        kc = f"c{g}_{l}"
                        c_ps = pspool.tile([128, 512], f32, tag="cb", bufs=int(os.environ.get("K_CB", "2")), name=f"cps{g}_{l}")
                        mm(kc, c_ps[:, 0:N], wt[l][:, COFF(l) : COFF(l) + 128], S["vt"][:, 0:N], start=True, stop=True)
                        mm(kc, c_ps[:, N : 2 * N], wt[l][:, COFF(l) + 128 : COFF(l) + 256], S["vt"][:, N : 2 * N], start=False, stop=True)
                        ct = mid.tile([128, 512], mt, tag="ct", bufs=4, name=f"ct{g}_{l}")
                        nc.scalar.activation(ct[:], c_ps[:], AF.Tanh)
                        kb = f"b{g}_{l}"
                        b_ps = pspool.tile([128, 512], f32, tag="cb", bufs=int(os.environ.get("K_CB", "2")), name=f"bps{g}_{l}")
                        mm(kb, b_ps[:, 0:N], Pb[:, 0:128], S["g4"][:], start=True, stop=True)
                        mm(kb, b_ps[:, N : 2 * N], Pb[:, 128:256], S["g4"][:], start=False, stop=True)
                        cg = mid.tile([128, 512], fp8, tag="cg", bufs=4, name=f"cg{g}_{l}")
                        nc.vector.tensor_mul(cg[:], ct[:], b_ps[:])
                        S["cg"] = cg
                        if l < L - 1:
                            S["xout"] = xpool.tile([128, NCH * N], mt, tag="xl", bufs=2 * int(os.environ.get("K_ROT", "4")), name=f"xl{g}_{l}")
                        else:
                            S["xout"] = xpool.tile([128, NCH * N], mt, tag="osb", bufs=int(os.environ.get("K_OSB", "4")), name=f"osb{g}")
                    # ---------- U matmuls (fp8 DoubleRow) + epilogue ----------
                    for g in gs:
                        S = st[g]
                        xin, x0t, cg, xout = S["xin"], S["x0t"], S["cg"], S["xout"]
                        cg2 = cg[:].rearrange("p (two n) -> p two n", two=2)
                        for q in range(4):
                            ku = f"u{g}_{l}_{q}"
                            u_ps = pspool.tile([128, 512], f32, tag="u", bufs=3, name=f"ups{g}_{l}_{q}")
                            for mi, mc in enumerate((2 * q, 2 * q + 1)):
                                mm(
                                    ku,
                                    u_ps[:, mi * N : (mi + 1) * N],
                                    wqt[l][
                                        :,
                                        (2048 if l == 0 else 0) + mc * 256 : (2048 if l == 0 else 0) + (mc + 1) * 256,
                                    ].rearrange("p (two m) -> p two m", two=2),
                                    cg2,
                                    start=(mi == 0),
                                    stop=(mi == 1),
                                    perf_mode=DR,
                                )
                            qs = slice(q * 2 * N, (q + 1) * 2 * N)
                            if bias_nonzero:
                                for mi, mc in enumerate((2 * q, 2 * q + 1)):
                                    col = mi * N
                                    nc.vector.scalar_tensor_tensor(
                                        xout[:, mc * N : (mc + 1) * N],
                                        u_ps[:, col : col + N],
                                        wb_f32[l][:, mc : mc + 1],
                                        x0t[:, mc * N : (mc + 1) * N],
                                        ALU.add,
                                        ALU.mult,
                                    )
                            elif q != 3:
                                # evacuate PSUM: xout_q = u_ps * x0 on DVE
                                nc.vector.tensor_mul(xout[:, qs], u_ps[:], x0t[:, qs])
                            else:
                                # ACT evacuates one quarter; Pool multiplies (SBUF-only)
                                usb = mid.tile([128, 512], f32, tag="usb", bufs=3, name=f"usb{g}_{l}_{q}")
                                nc.scalar.activation(usb[:], u_ps[:], AF.Copy)
                                nc.gpsimd.tensor_mul(xout[:, qs], usb[:], x0t[:, qs])
                            if q % 2 == 1:
                                hs = slice((q - 1) * 2 * N, (q + 1) * 2 * N)
                                add_eng = nc.gpsimd if (q == 3 and os.environ.get("K_POOLADD", "0") == "1") else nc.vector
                                add_eng.tensor_add(xout[:, hs], xout[:, hs], xin[:, hs])
                                if l < L - 1:
                                    nc.scalar.activation(xout[:, hs], xout[:, hs], AF.Tanh)
                                else:
                                    nc.sync.dma_start(
                                        outT_h[:, q // 2, :, g * N : (g + 1) * N],
                                        xout[:, hs].rearrange("p (c n) -> p c n", c=4),
                                    )
                    for g in gs:
                        st[g]["xin"] = st[g]["xout"]

    # walrus wait-budget legalization on serialization
    orig = nc.to_json_bytes
    nc.to_json_bytes = lambda: _legalize_json_bytes(orig())
    return nc


_CACHE = {}


MODE = "bf16"


def kernel(x, U, V, C, Wg, b):
    import ml_dtypes

    x = np.ascontiguousarray(np.asarray(x, np.float32))
    bias_nonzero = bool(np.any(np.asarray(b) != 0))
    key = ("nc", bias_nonzero, MODE)
    if key not in _CACHE:
        _CACHE[key] = build_nc(bias_nonzero, MODE)
    nc = _CACHE[key]
    mnp = ml_dtypes.bfloat16
    f8np = ml_dtypes.float8_e4m3
    blobs = pack_weights(U, V, C, Wg, b)
    xTfull = np.ascontiguousarray(x.T)  # [D, B]
    wls = {f"wl{l}": np.ascontiguousarray(blobs[f"wl{l}"].astype(mnp)) for l in range(L)}
    wqs = {f"wq{l}": np.ascontiguousarray(blobs[f"wq{l}"].astype(f8np)) for l in range(L)}
    wid = np.zeros((128, COLS_C), np.float32)
    wid[:, 0:128] = np.eye(128, dtype=np.float32)
    mcol = np.arange(256)
    wid[0:4, 128:384] = (np.arange(4)[:, None] == (mcol[None, :] // 64)).astype(np.float32)
    wid = wid.astype(mnp)
    wb = np.stack(
        [blobs[f"wl{l}"][:, (288 if l == 0 else 2336) : (288 if l == 0 else 2336) + NCH] for l in range(L)], axis=1
    ).reshape(128, L * NCH).astype(np.float32)
    in_maps = []
    for m in range(NCORES):
        shard = np.ascontiguousarray(xTfull[:, m * BC : (m + 1) * BC])
        im = {"wid": wid}
        im.update(wls)
        im.update(wqs)
        im["xT"] = shard.astype(mnp)
        im["xQ"] = shard.astype(f8np)
        if bias_nonzero:
            im["wb"] = wb
        in_maps.append(im)
    from concourse import bass2jax

    results = bass2jax.run_bass_via_pjrt(nc, in_maps, n_cores=NCORES)
    out = np.empty((B, D), np.float32)
    for m in range(NCORES):
        out[m * BC : (m + 1) * BC, :] = results[m]["outT"].T.astype(np.float32)
    return out


# revision 16
# speedup vs baseline: 1.0139x; 1.0078x over previous
"""CrossNetMoE forward on 8 Trainium2 NeuronCores (Bass/Tile).

Math (per layer i, E=4 experts, rank R=64, D=1024):
    v = tanh(V_e @ xl)            [B,E,R]
    c = tanh(C_e @ v_e)           [B,E,R]
    g = softmax(Wg_e . xl)        [B,E]
    u = sum_e (g_e * c_e) @ U_e.T + b      (softmax weights sum to 1)
    xl' = tanh(u * x0 + xl)   (last layer: no tanh)

Strategy: pure data-parallel over batch (2048 rows/core), transposed layout
[D, B] on-chip.  The U matmuls (all layers) and the V matmul of layer 0 run
as fp8e4 DoubleRow matmuls (two 128-deep k-tiles contracted per instruction
at 0.5 cycles/row); the moving operands come for free: layer-0 x is sent from
the host in fp8 alongside bf16, and cg (= tanh(c)*gate, all in (-1,1)) is
written in fp8 directly by the DVE multiply.  V matmuls of layers 1-2 stay
bf16 since xl only exists on-chip in bf16.  Softmax: one exp per group (no
ACT accumulator read), DVE segmented reduce for Z.  Epilogue: DVE multiplies
u (f32 PSUM) by x0 for quarters 0-1; quarters 2-3 are DMA-evacuated
PSUM->SBUF so the otherwise-idle Pool engine does those multiplies; the +xl
adds run on DVE at 4x rate (all-bf16, all-SBUF).
"""
import json
import os
import sys

sys.path.insert(0, "/opt/trn_rl_repo")

import numpy as np

L, E, D, R = 3, 4, 1024, 64
B = 16384
NCORES = 8
BC = B // NCORES          # 2048 rows per core
N = 256                   # batch columns per group (matmul moving free dim)
G = BC // N               # 8 groups per core
NCH = D // 128            # 8 d-chunks

# bf16 blob layout (per layer)
# l0:  [C 256 | W 32 | b 8]                      = 296 cols
# l1+: [V 2048 | C 256 | W 32 | b 8]             = 2344 cols
COLS_L0 = 296
COLS_L12 = 2344
# fp8 blob layout: l0: [Vpair 2048 | Upair 2048]; l1+: [Upair 2048]
QCOLS_L0 = 4096
QCOLS_L12 = 2048
# common blob
I_OFF = 0                 # identity 128
P_OFF = 128               # 2 groups x 128 (partitions 0..3)
COLS_C = 388

_EXEMPT = {"Call"}


def _legalize_json_bytes(raw: bytes) -> bytes:
    """Split multi-wait instructions: walrus allows 1 sync-wait per inst."""
    m = json.loads(raw)
    counter = [0]

    def fix_block(block):
        insts = block.get("instructions")
        if insts is not None:
            out = []
            for inst in insts:
                si = inst.get("sync_info")
                if (
                    si
                    and inst.get("opcode") not in _EXEMPT
                    and len(si.get("on_wait") or []) > 1
                ):
                    for w in si["on_wait"][:-1]:
                        counter[0] += 1
                        out.append(
                            {
                                "name": f"I-waitsplit-{counter[0]}",
                                "opcode": "NoOp",
                                "engine": inst["engine"],
                                "ins": [],
                                "outs": [],
                                "debug": 0,
                                "sync_info": {"on_wait": [w], "on_update": []},
                            }
                        )
                    si["on_wait"] = [si["on_wait"][-1]]
                out.append(inst)
            block["instructions"] = out
        for sub in block.get("blocks") or []:
            fix_block(sub)

    for f in m["functions"]:
        for b in f["blocks"]:
            fix_block(b)
    return json.dumps(m).encode()


def _v_pack(V, l):
    """Baseline V chunk pack: [p=d%128, (eg, c, m)] m->(e=2eg+m//64, r=m%64)."""
    p = np.arange(128)
    m = np.arange(128)
    out = np.zeros((128, 2048), np.float32)
    for g in range(2):
        for c in range(NCH):
            out[:, (g * 8 + c) * 128 : (g * 8 + c + 1) * 128] = V[
                l, 2 * g + m[None, :] // 64, m[None, :] % 64, c * 128 + p[:, None]
            ]
    return out


def _u_pack(U, l):
    """Baseline U pack [p=(e,r2)%128 of kchunk, (kc, mc, q)]."""
    p = np.arange(128)
    q = np.arange(128)
    out = np.zeros((128, 2048), np.float32)
    for kc in range(2):
        for mc in range(NCH):
            out[:, (kc * 8 + mc) * 128 : (kc * 8 + mc + 1) * 128] = U[
                l, (kc * 128 + p[:, None]) // 64, mc * 128 + q[None, :],
                (kc * 128 + p[:, None]) % 64,
            ]
    return out


def pack_weights(U, V, C, Wg, b):
    """Pack parameters into SBUF-image blobs (host side)."""
    U, V, C, Wg, b = (np.asarray(a, np.float32) for a in (U, V, C, Wg, b))
    blobs = {}
    p = np.arange(128)
    for l in range(L):
        cols = COLS_L0 if l == 0 else COLS_L12
        voff = 0 if l == 0 else 2048
        blob = np.zeros((128, cols), np.float32)
        if l > 0:
            blob[:, 0:2048] = _v_pack(V, l)
        # Cw: blockdiag pairs: [p=(el',r1), (g,j=(el,r2))]
        for g in range(2):
            j = np.arange(128)
            el_p = p[:, None] // 64
            el_j = j[None, :] // 64
            val = C[l, 2 * g + el_j, j[None, :] % 64, p[:, None] % 64]
            blob[:, voff + g * 128 : voff + (g + 1) * 128] = np.where(
                el_p == el_j, val, 0.0
            )
        # Ww: [p=d%128, (c,e)]
        for c in range(NCH):
            blob[:, voff + 256 + c * 4 : voff + 256 + (c + 1) * 4] = Wg[
                l, :, c * 128 + p
            ]
        for c in range(NCH):
            blob[:, voff + 288 + c] = b[l, c * 128 + p]
        blobs[f"wl{l}"] = blob
        # fp8 blob: U DoubleRow pairs: for mc: [ktileA=Upack(kc0,mc) | ktileB=Upack(kc1,mc)]
        up = _u_pack(U, l)
        qcols = QCOLS_L0 if l == 0 else QCOLS_L12
        qb = np.zeros((128, qcols), np.float32)
        uoff = 2048 if l == 0 else 0
        for mc in range(NCH):
            qb[:, uoff + mc * 256 : uoff + mc * 256 + 128] = up[:, mc * 128 : (mc + 1) * 128]
            qb[:, uoff + mc * 256 + 128 : uoff + mc * 256 + 256] = up[
                :, (8 + mc) * 128 : (9 + mc) * 128
            ]
        if l == 0:
            vp = _v_pack(V, 0)
            for g in range(2):
                for cp in range(4):
                    base = (g * 4 + cp) * 256
                    qb[:, base : base + 128] = vp[
                        :, (g * 8 + 2 * cp) * 128 : (g * 8 + 2 * cp + 1) * 128
                    ]
                    qb[:, base + 128 : base + 256] = vp[
                        :, (g * 8 + 2 * cp + 1) * 128 : (g * 8 + 2 * cp + 2) * 128
                    ]
        blobs[f"wq{l}"] = qb
    wc = np.zeros((128, COLS_C), np.float32)
    wc[:, 0:128] = np.eye(128, dtype=np.float32)
    for g in range(2):
        m = np.arange(128)
        wc[0:4, P_OFF + g * 128 : P_OFF + (g + 1) * 128] = (
            np.arange(4)[:, None] == (2 * g + m[None, :] // 64)
        ).astype(np.float32)
    blobs["wc"] = wc
    return blobs


def build_nc(bias_nonzero=False, mode="bf16"):
    import concourse.bass as bass
    import concourse.tile as tile
    from concourse import mybir
    from concourse.tile import add_dep_helper

    f32 = mybir.dt.float32
    AF = mybir.ActivationFunctionType
    ALU = mybir.AluOpType
    bf16 = mybir.dt.bfloat16
    fp8 = mybir.dt.float8e4
    DR = mybir.MatmulPerfMode.DoubleRow
    mt = bf16

    nc = bass.Bass()
    xT = nc.dram_tensor("xT", [D, BC], mt, kind="ExternalInput")
    xQ = nc.dram_tensor("xQ", [D, BC], fp8, kind="ExternalInput")
    wl = [
        nc.dram_tensor(f"wl{l}", [128, COLS_L0 if l == 0 else COLS_L12], mt,
                       kind="ExternalInput")
        for l in range(L)
    ]
    wq = [
        nc.dram_tensor(f"wq{l}", [128, QCOLS_L0 if l == 0 else QCOLS_L12], fp8,
                       kind="ExternalInput")
        for l in range(L)
    ]
    wbd = nc.dram_tensor("wb", [128, L * NCH], f32, kind="ExternalInput") if bias_nonzero else None
    wid = nc.dram_tensor("wid", [128, COLS_C], mt, kind="ExternalInput")
    outT = nc.dram_tensor("outT", [D, BC], mt, kind="ExternalOutput")

    # chain matmuls that share a psum tile so scheduler keeps program order
    last_mm = {}

    def mm(key, out, lhsT, rhs, start, stop, perf_mode=None):
        inst = nc.tensor.matmul(
            out, lhsT, rhs, start=start, stop=stop, skip_group_check=True,
            perf_mode=perf_mode,
        )
        if key in last_mm:
            add_dep_helper(inst.ins, last_mm[key].ins, sync=False, reason="psum order")
        last_mm[key] = inst
        return inst

    with tile.TileContext(nc) as tc:
        with (
            tc.tile_pool(name="wpool", bufs=1) as wpool,
            tc.tile_pool(name="xpool", bufs=1) as xpool,
            tc.tile_pool(name="mid", bufs=1) as mid,
            tc.tile_pool(name="pspool", bufs=1, space="PSUM") as pspool,
            nc.allow_low_precision(reason="bf16/fp8 matmul pipeline (intentional)"),
        ):
            wt = []
            wqt = []
            for l in range(L):
                wt.append(wpool.tile([128, COLS_L0 if l == 0 else COLS_L12], mt, name=f"wt{l}"))
                wqt.append(wpool.tile([128, QCOLS_L0 if l == 0 else QCOLS_L12], fp8, name=f"wq{l}"))
            wb_f32 = None
            identm = wpool.tile([128, COLS_C], mt, name="identm")
            Pb = identm[0:4, 128:384]

            def VOFF(l):
                return 0 if l == 0 else 0  # V cols start at 0 for l>=1
            def COFF(l):
                return 0 if l == 0 else 2048
            def WOFF(l):
                return 256 if l == 0 else 2304
            def BOFF(l):
                return 288 if l == 0 else 2336

            xT_v = xT.rearrange("(c p) b -> p c b", p=128)
            xQ_v = xQ.rearrange("(c p) b -> p c b", p=128)
            outT_h = outT.rearrange("(h c p) b -> p h c b", p=128, h=2)

            ROT = int(os.environ.get('K_ROT', '4'))  # groups processed in rotation
            for sup in range(G // ROT):
                gs = [ROT * sup + i for i in range(ROT)]
                st = {}
                if sup == 0:
                    # startup: fan the critical tiles across 4 DMA queues so
                    # layer-0 compute starts as early as possible
                    qs_ = [nc.sync, nc.gpsimd, nc.scalar]
                    # group-0 fp8 x + V0 fp8 weights first (v matmul l0)
                    x0t = xpool.tile([128, NCH * N], mt, tag="x0", bufs=8, name="x0_0")
                    x0q = xpool.tile([128, NCH * N], fp8, tag="xq", bufs=int(os.environ.get("K_ROT", "4")), name="xq_0")
                    nc.sync.dma_start(x0q[:], xQ_v[:, :, 0:N])
                    nc.gpsimd.dma_start(wqt[0][:, 0:2048], wq[0][:, 0:2048])
                    nc.scalar.dma_start(x0t[:], xT_v[:, :, 0:N])
                    nc.gpsimd.dma_start(wt[0][:], wl[0][:])
                    nc.gpsimd.dma_start(identm[:], wid[:])
                    st[gs[0]] = dict(x0t=x0t, x0q=x0q, xin=x0t)
                    for i, g in enumerate(gs[1:]):
                        x0t = xpool.tile([128, NCH * N], mt, tag="x0", bufs=8, name=f"x0_{g}")
                        x0q = xpool.tile([128, NCH * N], fp8, tag="xq", bufs=int(os.environ.get("K_ROT", "4")), name=f"xq_{g}")
                        qs_[i % 3].dma_start(x0q[:], xQ_v[:, :, g * N : (g + 1) * N])
                        qs_[(i + 1) % 3].dma_start(x0t[:], xT_v[:, :, g * N : (g + 1) * N])
                        st[g] = dict(x0t=x0t, x0q=x0q, xin=x0t)
                    # U0 fp8 + remaining layers, round-robin across queues
                    nc.sync.dma_start(wqt[0][:, 2048:4096], wq[0][:, 2048:4096])
                    if bias_nonzero:
                        wbt = wpool.tile([128, L * NCH], f32)
                        nc.sync.dma_start(wbt[:], wbd[:])
                        wb_f32 = [wbt[:, l * NCH : (l + 1) * NCH] for l in range(L)]
                    for l_ in range(1, L):
                        qs_[(l_ * 2) % 3].dma_start(wt[l_][:, 0:1172], wl[l_][:, 0:1172])
                        qs_[(l_ * 2 + 1) % 3].dma_start(wt[l_][:, 1172:COLS_L12], wl[l_][:, 1172:COLS_L12])
                        qs_[(l_ * 2 + 2) % 3].dma_start(wqt[l_][:], wq[l_][:])
                else:
                    for i, g in enumerate(gs):
                        x0t = xpool.tile([128, NCH * N], mt, tag="x0", bufs=8, name=f"x0_{g}")
                        x0q = xpool.tile([128, NCH * N], fp8, tag="xq", bufs=int(os.environ.get("K_ROT", "4")), name=f"xq_{g}")
                        nc.sync.dma_start(x0t[:], xT_v[:, :, g * N : (g + 1) * N])
                        nc.scalar.dma_start(x0q[:], xQ_v[:, :, g * N : (g + 1) * N])
                        st[g] = dict(x0t=x0t, x0q=x0q, xin=x0t)

                for l in range(L):
                    # ---------- P1 + per-group softmax chain ----------
                    for gi, g in enumerate(gs):
                        S = st[g]
                        xin = S["xin"]
                        # gate logits first: the softmax chain is the longest
                        # dependency chain, start it as early as possible
                        ks = f"sb{g}_{l}"
                        sbt = pspool.tile([128, 8], f32, tag="sbt", bufs=int(os.environ.get("K_SBT", "1")), name=f"sbt{g}_{l}")
                        for h in range(2):
                            for c in range(NCH):
                                mm(
                                    ks,
                                    sbt[:, h * 4 : (h + 1) * 4],
                                    xin[:, c * N + h * 128 : c * N + h * 128 + 128],
                                    wt[l][:, WOFF(l) + c * 4 : WOFF(l) + (c + 1) * 4],
                                    start=(c == 0),
                                    stop=(c == NCH - 1),
                                )
                        ebt = mid.tile([128, 8], f32, tag="ebt", bufs=5, name=f"ebt{g}_{l}")
                        nc.scalar.activation(ebt[:], sbt[:], AF.Exp)
                        kv = f"v{g}_{l}"
                        v_ps = pspool.tile([128, 512], f32, tag="v", bufs=int(os.environ.get("K_V", "1")), name=f"vps{g}_{l}")
                        if l == 0:
                            xq = S["x0q"]
                            for eg in range(2):
                                for cp in range(4):
                                    mm(
                                        kv,
                                        v_ps[:, eg * N : (eg + 1) * N],
                                        wqt[0][:, (eg * 4 + cp) * 256 : (eg * 4 + cp + 1) * 256]
                                        .rearrange("p (two m) -> p two m", two=2),
                                        xq[:, cp * 512 : (cp + 1) * 512]
                                        .rearrange("p (two n) -> p two n", two=2),
                                        start=(cp == 0),
                                        stop=(cp == 3),
                                        perf_mode=DR,
                                    )
                        else:
                            for c in range(NCH):
                                rhs = xin[:, c * N : (c + 1) * N]
                                mm(kv, v_ps[:, 0:N], wt[l][:, c * 128 : (c + 1) * 128], rhs, start=(c == 0), stop=False)
                                mm(kv, v_ps[:, N : 2 * N], wt[l][:, (8 + c) * 128 : (9 + c) * 128], rhs, start=False, stop=(c == NCH - 1))
                        # chain: tanh(v); softmax over the 4-col free dim
                        vt = mid.tile([128, 512], mt, tag="vt", bufs=5, name=f"vt{g}_{l}")
                        nc.scalar.activation(vt[:], v_ps[:], AF.Tanh)
                        zsum = mid.tile([128, 2], f32, tag="zs", bufs=5, name=f"zs{g}_{l}")
                        nc.vector.tensor_reduce(
                            zsum[:],
                            ebt[:].rearrange("p (g x) -> p g x", g=2),
                            mybir.AxisListType.X,
                            ALU.add,
                        )
                        rz = mid.tile([128, 2], f32, tag="rz", bufs=5, name=f"rz{g}_{l}")
                        nc.vector.reciprocal(rz[:], zsum[:])
                        gbt = mid.tile([128, 8], mt, tag="gbt", bufs=5, name=f"gbt{g}_{l}")
                        for h in range(2):
                            nc.vector.tensor_scalar_mul(
                                gbt[:, h * 4 : (h + 1) * 4],
                                ebt[:, h * 4 : (h + 1) * 4],
                                rz[:, h : h + 1],
                            )
                        # transpose gates back to [expert, batch] (bf16 PSUM out)
                        kt = f"gt{g}_{l}"
                        gt = pspool.tile([4, 256], mt, tag="gt", bufs=int(os.environ.get("K_GT", "1")), name=f"gt{g}_{l}")
                        for h in range(2):
                            inst = nc.tensor.transpose(
                                gt[0:4, h * 128 : (h + 1) * 128],
                                gbt[:, h * 4 : (h + 1) * 4],
                                identm[:, 0:128],
                            )
                            if kt in last_mm:
                                add_dep_helper(inst.ins, last_mm[kt].ins, sync=False, reason="psum order")
                            last_mm[kt] = inst
                        g4 = mid.tile([4, N], mt, tag="g4", bufs=5, name=f"g4{g}_{l}")
                        nc.vector.tensor_copy(g4[:], gt[0:4, :])
                        S["vt"], S["g4"] = vt, g4
                    # ---------- C / gate-broadcast / cg ----------
                    for g in gs:
                        S = st[g]
                        kc = f"c{g}_{l}"
                        c_ps = pspool.tile([128, 512], f32, tag="cb", bufs=int(os.environ.get("K_CB", "2")), name=f"cps{g}_{l}")
                        mm(kc, c_ps[:, 0:N], wt[l][:, COFF(l) : COFF(l) + 128], S["vt"][:, 0:N], start=True, stop=True)
                        mm(kc, c_ps[:, N : 2 * N], wt[l][:, COFF(l) + 128 : COFF(l) + 256], S["vt"][:, N : 2 * N], start=False, stop=True)
                        ct = mid.tile([128, 512], mt, tag="ct", bufs=4, name=f"ct{g}_{l}")
                        nc.scalar.activation(ct[:], c_ps[:], AF.Tanh)
                        kb = f"b{g}_{l}"
                        b_ps = pspool.tile([128, 512], f32, tag="cb", bufs=int(os.environ.get("K_CB", "2")), name=f"bps{g}_{l}")
                        mm(kb, b_ps[:, 0:N], Pb[:, 0:128], S["g4"][:], start=True, stop=True)
                        mm(kb, b_ps[:, N : 2 * N], Pb[:, 128:256], S["g4"][:], start=False, stop=True)
                        cg = mid.tile([128, 512], fp8, tag="cg", bufs=4, name=f"cg{g}_{l}")
                        nc.vector.tensor_mul(cg[:], ct[:], b_ps[:])
                        S["cg"] = cg
                        if l < L - 1:
                            S["xout"] = xpool.tile([128, NCH * N], mt, tag="xl", bufs=2 * int(os.environ.get("K_ROT", "4")), name=f"xl{g}_{l}")
                        else:
                            S["xout"] = xpool.tile([128, NCH * N], mt, tag="osb", bufs=int(os.environ.get("K_OSB", "4")), name=f"osb{g}")
                    # ---------- U matmuls (fp8 DoubleRow) + epilogue ----------
                    for g in gs:
                        S = st[g]
                        xin, x0t, cg, xout = S["xin"], S["x0t"], S["cg"], S["xout"]
                        cg2 = cg[:].rearrange("p (two n) -> p two n", two=2)
                        for q in range(4):
                            ku = f"u{g}_{l}_{q}"
                            u_ps = pspool.tile([128, 512], f32, tag="u", bufs=3, name=f"ups{g}_{l}_{q}")
                            for mi, mc in enumerate((2 * q, 2 * q + 1)):
                                mm(
                                    ku,
                                    u_ps[:, mi * N : (mi + 1) * N],
                                    wqt[l][
                                        :,
                                        (2048 if l == 0 else 0) + mc * 256 : (2048 if l == 0 else 0) + (mc + 1) * 256,
                                    ].rearrange("p (two m) -> p two m", two=2),
                                    cg2,
                                    start=(mi == 0),
                                    stop=(mi == 1),
                                    perf_mode=DR,
                                )
                            qs = slice(q * 2 * N, (q + 1) * 2 * N)
                            if bias_nonzero:
                                for mi, mc in enumerate((2 * q, 2 * q + 1)):
                                    col = mi * N
                                    nc.vector.scalar_tensor_tensor(
                                        xout[:, mc * N : (mc + 1) * N],
                                        u_ps[:, col : col + N],
                                        wb_f32[l][:, mc : mc + 1],
                                        x0t[:, mc * N : (mc + 1) * N],
                                        ALU.add,
                                        ALU.mult,
                                    )
                            elif os.environ.get("K_Q3DVE", "0") == "1" or (q != 3 and not (q == 2 and os.environ.get("K_Q2EVAC", "0") == "1")):
                                # evacuate PSUM: xout_q = u_ps * x0 on DVE
                                nc.vector.tensor_mul(xout[:, qs], u_ps[:], x0t[:, qs])
                            else:
                                # ACT evacuates one quarter; Pool multiplies (SBUF-only)
                                usb = mid.tile([128, 512], f32, tag="usb", bufs=3, name=f"usb{g}_{l}_{q}")
                                nc.scalar.activation(usb[:], u_ps[:], AF.Copy)
                                nc.gpsimd.tensor_mul(xout[:, qs], usb[:], x0t[:, qs])
                            if q % 2 == 1:
                                hs = slice((q - 1) * 2 * N, (q + 1) * 2 * N)
                                if os.environ.get("K_STTADD", "0") == "1":
                                    # TensorScalarPtr supports the DVE 4x mode
                                    nc.vector.scalar_tensor_tensor(
                                        xout[:, hs], xout[:, hs], 0.0, xin[:, hs],
                                        ALU.add, ALU.add,
                                    )
                                else:
                                    nc.vector.tensor_add(xout[:, hs], xout[:, hs], xin[:, hs])
                                if l < L - 1:
                                    nc.scalar.activation(xout[:, hs], xout[:, hs], AF.Tanh)
                                else:
                                    nc.sync.dma_start(
                                        outT_h[:, q // 2, :, g * N : (g + 1) * N],
                                        xout[:, hs].rearrange("p (c n) -> p c n", c=4),
                                    )
                    for g in gs:
                        st[g]["xin"] = st[g]["xout"]

    # walrus wait-budget legalization on serialization
    orig = nc.to_json_bytes
    nc.to_json_bytes = lambda: _legalize_json_bytes(orig())
    return nc


_CACHE = {}


MODE = "bf16"


def kernel(x, U, V, C, Wg, b):
    import ml_dtypes

    x = np.ascontiguousarray(np.asarray(x, np.float32))
    bias_nonzero = bool(np.any(np.asarray(b) != 0))
    key = ("nc", bias_nonzero, MODE)
    if key not in _CACHE:
        _CACHE[key] = build_nc(bias_nonzero, MODE)
    nc = _CACHE[key]
    mnp = ml_dtypes.bfloat16
    f8np = ml_dtypes.float8_e4m3
    blobs = pack_weights(U, V, C, Wg, b)
    xTfull = np.ascontiguousarray(x.T)  # [D, B]
    wls = {f"wl{l}": np.ascontiguousarray(blobs[f"wl{l}"].astype(mnp)) for l in range(L)}
    wqs = {f"wq{l}": np.ascontiguousarray(blobs[f"wq{l}"].astype(f8np)) for l in range(L)}
    wid = np.zeros((128, COLS_C), np.float32)
    wid[:, 0:128] = np.eye(128, dtype=np.float32)
    mcol = np.arange(256)
    wid[0:4, 128:384] = (np.arange(4)[:, None] == (mcol[None, :] // 64)).astype(np.float32)
    wid = wid.astype(mnp)
    wb = np.stack(
        [blobs[f"wl{l}"][:, (288 if l == 0 else 2336) : (288 if l == 0 else 2336) + NCH] for l in range(L)], axis=1
    ).reshape(128, L * NCH).astype(np.float32)
    in_maps = []
    for m in range(NCORES):
        shard = np.ascontiguousarray(xTfull[:, m * BC : (m + 1) * BC])
        im = {"wid": wid}
        im.update(wls)
        im.update(wqs)
        im["xT"] = shard.astype(mnp)
        im["xQ"] = shard.astype(f8np)
        if bias_nonzero:
            im["wb"] = wb
        in_maps.append(im)
    from concourse import bass2jax

    results = bass2jax.run_bass_via_pjrt(nc, in_maps, n_cores=NCORES)
    out = np.empty((B, D), np.float32)
    for m in range(NCORES):
        out[m * BC : (m + 1) * BC, :] = results[m]["outT"].T.astype(np.float32)
    return out
